# revision 15
# baseline (speedup 1.0000x reference)
"""Ball-query kernel for Trainium2 (Bass/Tile), 8 NeuronCores.

Problem: for each batch b (8 total) and each query point m (4096), return the
first 32 source indices n (in increasing n) with ||q_m - p_n||^2 < 0.2^2,
padding unused slots with the first valid index. Queries == sources (xyz).

Sharding: data-parallel over batch, one batch per core (8 cores).

Per-core algorithm (N=4096 queries x 4096 sources):
  - PE computes dot[m, n] = q_m . p_n per 128-query block (K=3 matmul).
  - DVE scalar_tensor_tensor: s = 2*dot - plus, where plus[m,n] = sq[m]+sq[n]
    (s == -d2 with bit-exact match to the reference's rounding order).
  - DVE STT: keys = (s > -r^2) * (4096 - n)  -> valid keys descending encode
    ascending indices; invalid -> 0.
  - 4 rounds of vector.max (top-8, descending) + match_replace to extract the
    32 largest keys = first 32 valid indices, in order.
  - Pad empty slots (key 0) with the first valid key; idx = 4096 - key.

Host I/O is the bottleneck (axon-tunneled PJRT), so the host->device payload
is a single [4, 4096] f32 tensor per core (xyzT rows + sq row; sqq and the
[128, N] broadcasts are materialized on-device via rearranged/stride-0 DMA +
iota), index pairs travel back packed as 24-bit triples in a uint8 tensor
(idx_even + 4096*idx_odd, exact in f32 since it is < 2^24), and the jitted
executable + donated output buffer are cached across kernel() calls.
"""

import numpy as np

N = 4096
NS = 32
R2 = 0.04
NCORES = 8
BLK = 128
NBLK = N // BLK   # 32
CH = 2048         # dve chunk
NCH = N // CH     # 2
MM = 512          # matmul free-dim per instruction (1 bank)
NP2 = NS // 2     # 16 packed index pairs per query
NB3 = 3 * NP2     # 48 output bytes per query

_CACHE = {}


def _build_bass():
    import concourse.bass as bass
    import concourse.mybir as mybir
    from concourse import bacc, tile

    Alu = mybir.AluOpType
    f32 = mybir.dt.float32
    i32 = mybir.dt.int32
    u8 = mybir.dt.uint8

    nc = bacc.Bacc(
        "TRN2", target_bir_lowering=False, debug=False, num_devices=NCORES
    )

    # xyzT only; sq / sqq / broadcasts are derived on-device
    inp_d = nc.dram_tensor("inp", [3, N], f32, kind="ExternalInput")
    out_d = nc.dram_tensor("out", [N, NB3], u8, kind="ExternalOutput")

    with tile.TileContext(nc) as tc:
        with (
            tc.tile_pool(name="const", bufs=1) as cpool,
            tc.tile_pool(name="psum", bufs=8, space="PSUM") as ppool,
            tc.tile_pool(name="work", bufs=2) as wpool,
            tc.tile_pool(name="small", bufs=3) as spool,
        ):
            xyzT_sb = cpool.tile([3, N], f32, tag="xyzT", name="xyzT_sb")
            nc.gpsimd.dma_start(xyzT_sb[:], inp_d.ap())
            # block layout xyzB[p, 3b+c] = xyz[b*128+p, c] for per-query sq
            xyzB = cpool.tile([128, 3 * NBLK], f32, tag="xyzB", name="xyzB")
            for c in range(3):
                nc.gpsimd.dma_start(
                    xyzB[:, c::3],
                    inp_d.ap()[c : c + 1, :].rearrange("a (b p) -> p (a b)", p=128),
                )
            # sqq[p, b] = (x^2 + y^2) + z^2 of query b*128+p (IEEE order matches
            # the reference's jnp.sum on CPU)
            sqq_sb = cpool.tile([128, NBLK], f32, tag="sqq", name="sqq_sb")
            sxx = spool.tile([128, NBLK], f32, tag="sxx", name="sxx")
            syy = spool.tile([128, NBLK], f32, tag="syy", name="syy")
            szz = spool.tile([128, NBLK], f32, tag="szz", name="szz")
            nc.vector.tensor_tensor(sxx[:], xyzB[:, 0::3], xyzB[:, 0::3], Alu.mult)
            nc.vector.tensor_tensor(syy[:], xyzB[:, 1::3], xyzB[:, 1::3], Alu.mult)
            nc.vector.tensor_tensor(szz[:], xyzB[:, 2::3], xyzB[:, 2::3], Alu.mult)
            nc.vector.tensor_tensor(sxx[:], sxx[:], syy[:], Alu.add)
            nc.vector.tensor_tensor(sqq_sb[:], sxx[:], szz[:], Alu.add)
            # sqrep[p, n] = sq[n]: ones-matmul over squared coords (PE k-order
            # accumulation is bit-identical to the CPU einsum, as the dot
            # products below rely on already)
            xyzT2 = cpool.tile([3, N], f32, tag="xyzT2", name="xyzT2")
            nc.gpsimd.tensor_tensor(xyzT2[:], xyzT_sb[:], xyzT_sb[:], Alu.mult)
            ones3 = cpool.tile([3, 128], f32, tag="ones3", name="ones3")
            nc.gpsimd.memset(ones3[:], 1.0)
            sqrep = cpool.tile([128, N], f32, tag="sqrep", name="sqrep")
            for j in range(N // MM):
                ps = ppool.tile([128, MM], f32, tag="ps", name="ps")
                nc.tensor.matmul(
                    ps[:],
                    ones3[:],
                    xyzT2[:, j * MM : (j + 1) * MM],
                    start=True,
                    stop=True,
                )
                nc.vector.tensor_copy(sqrep[:, j * MM : (j + 1) * MM], ps[:])
            # ineg[p, n] = 4096 - n (exact in f32)
            ineg = cpool.tile([128, N], f32, tag="ineg", name="ineg")
            nc.gpsimd.iota(
                ineg[:],
                pattern=[[-1, N]],
                base=N,
                channel_multiplier=0,
                allow_small_or_imprecise_dtypes=True,
            )

            for b in range(NBLK):
                # plus[m, n] = sq_q[m] + sq_src[n]
                plus = wpool.tile([128, N], f32, tag="plus", name="plus")
                for c in range(NCH):
                    nc.vector.tensor_scalar(
                        plus[:, c * CH : (c + 1) * CH],
                        sqrep[:, c * CH : (c + 1) * CH],
                        sqq_sb[:, b : b + 1],
                        None,
                        Alu.add,
                    )

                keys = wpool.tile([128, N], f32, tag="keys", name="keys")
                keys2 = wpool.tile([128, N], f32, tag="keys2", name="keys2")

                for j in range(N // MM):
                    ps = ppool.tile([128, MM], f32, tag="ps", name="ps")
                    nc.tensor.matmul(
                        ps[:],
                        xyzT_sb[:, b * BLK : (b + 1) * BLK],
                        xyzT_sb[:, j * MM : (j + 1) * MM],
                        start=True,
                        stop=True,
                    )
                    # s = 2*dot - plus  (== -d2, exact)
                    nc.vector.scalar_tensor_tensor(
                        keys2[:, j * MM : (j + 1) * MM],
                        ps[:],
                        2.0,
                        plus[:, j * MM : (j + 1) * MM],
                        Alu.mult,
                        Alu.subtract,
                    )
                for c in range(NCH):
                    # keys = (s > -r2) * (4096 - n)
                    nc.vector.scalar_tensor_tensor(
                        keys[:, c * CH : (c + 1) * CH],
                        keys2[:, c * CH : (c + 1) * CH],
                        -R2,
                        ineg[:, c * CH : (c + 1) * CH],
                        Alu.is_gt,
                        Alu.mult,
                    )

                v8 = spool.tile([128, NS], f32, tag="v8", name="v8")
                nc.vector.max(v8[:, 0:8], keys[:])
                nc.vector.match_replace(keys2[:], v8[:, 0:8], keys[:], 0.0)
                nc.vector.max(v8[:, 8:16], keys2[:])
                nc.vector.match_replace(keys[:], v8[:, 8:16], keys2[:], 0.0)
                nc.vector.max(v8[:, 16:24], keys[:])
                nc.vector.match_replace(keys2[:], v8[:, 16:24], keys[:], 0.0)
                nc.vector.max(v8[:, 24:32], keys2[:])

                # pad empty slots (0) with first valid key, then pack index
                # pairs: idx_even + 4096*idx_odd = 2^24 + 4096 - k_e - 4096*k_o
                # (exact in f32: result <= 2^24 - 1)
                f8 = spool.tile([128, NS], f32, tag="f8", name="f8")
                nc.vector.tensor_scalar(f8[:], v8[:], 0.0, None, Alu.is_equal)
                t2 = spool.tile([128, NS], f32, tag="t2", name="t2")
                nc.vector.scalar_tensor_tensor(
                    t2[:], f8[:], v8[:, 0:1], v8[:], Alu.mult, Alu.add
                )
                idxf = spool.tile([128, NS], f32, tag="idxf", name="idxf")
                nc.vector.tensor_scalar(
                    idxf[:], t2[:], -1.0, float(N), Alu.mult, Alu.add
                )
                packed = spool.tile([128, NP2], i32, tag="packed", name="packed")
                nc.vector.scalar_tensor_tensor(
                    packed[:], idxf[:, NP2:NS], 4096.0, idxf[:, 0:NP2],
                    Alu.mult, Alu.add,
                )
                # byte-plane output (LE): cols 0:32 = lo16 pairs, 32:48 = hi bytes
                src_b = packed[:].bitcast(u8).rearrange("p (j k) -> p j k", k=4)
                orow = out_d.ap()[b * BLK : (b + 1) * BLK, :]
                nc.sync.dma_start(
                    orow[:, 0 : 2 * NP2].rearrange("p (j k) -> p j k", k=2),
                    src_b[:, :, 0:2],
                )
                nc.sync.dma_start(orow[:, 2 * NP2 : NB3], src_b[:, :, 2])

    nc.compile()
    return nc


def _init():
    import jax
    from jax.sharding import Mesh, PartitionSpec, NamedSharding

    try:
        from jax.experimental.shard_map import shard_map
    except ImportError:
        from jax import shard_map
    import concourse.mybir as mybir
    from concourse.bass2jax import (
        _bass_exec_p,
        install_neuronx_cc_hook,
        partition_id_tensor,
    )

    install_neuronx_cc_hook()
    nc = _build_bass()

    partition_name = (
        nc.partition_id_tensor.name if nc.partition_id_tensor else None
    )
    in_names, out_names, out_avals = [], [], []
    for alloc in nc.m.functions[0].allocations:
        if not isinstance(alloc, mybir.MemoryLocationSet):
            continue
        name = alloc.memorylocations[0].name
        if alloc.kind == "ExternalInput":
            if name != partition_name:
                in_names.append(name)
        elif alloc.kind == "ExternalOutput":
            shape = tuple(alloc.tensor_shape)
            dtype = mybir.dt.np(alloc.dtype)
            out_names.append(name)
            out_avals.append(jax.core.ShapedArray(shape, dtype))
    n_params = len(in_names)
    n_outs = len(out_avals)
    in_names_full = in_names + out_names + (
        [partition_name] if partition_name else []
    )
    donate = tuple(range(n_params, n_params + n_outs))

    def _body(*args):
        operands = list(args)
        if partition_name is not None:
            operands.append(partition_id_tensor())
        outs = _bass_exec_p.bind(
            *operands,
            out_avals=tuple(out_avals),
            in_names=tuple(in_names_full),
            out_names=tuple(out_names),
            lowering_input_output_aliases=(),
            sim_require_finite=True,
            sim_require_nnan=True,
            nc=nc,
        )
        return tuple(outs)

    devices = jax.devices()[:NCORES]
    mesh = Mesh(np.asarray(devices), ("core",))
    sh = NamedSharding(mesh, PartitionSpec("core"))
    in_specs = (PartitionSpec("core"),) * (n_params + n_outs)
    out_specs = (PartitionSpec("core"),) * n_outs
    fn = jax.jit(
        shard_map(
            _body, mesh=mesh, in_specs=in_specs, out_specs=out_specs,
            check_rep=False,
        ),
        donate_argnums=donate,
        keep_unused=True,
    )
    _CACHE.update(
        jax=jax, fn=fn, sh=sh, prev=None,
        out_shape=[(NCORES * N, NB3)], out_dtype=[np.uint8],
    )
    return _CACHE


def _prep(xyz):
    # [8, 4096, 3] -> concat of per-core xyzT [3, 4096]
    xyz = np.asarray(xyz, dtype=np.float32)
    return np.ascontiguousarray(xyz.transpose(0, 2, 1)).reshape(NCORES * 3, N)


def _unpack(res):
    # uint8 [8*4096, 48]: cols 0:32 = LE lo16 of pair j (idx_j + 4096*idx_{j+16}),
    # cols 32:48 = hi byte -> int32 [8, 4096, 32]
    lo = res[:, 0 : 2 * NP2].view(np.uint16)
    hi = res[:, 2 * NP2 : NB3]
    out = np.empty((NCORES, N, NS), np.int32)
    o = out.reshape(NCORES * N, NS)
    o[:, 0:NP2] = lo & 0xFFF
    o[:, NP2:NS] = (lo >> 12) | (hi.astype(np.uint16) << 4)
    return out


def kernel(xyz, xyz_new=None):
    st = _CACHE if _CACHE else _init()
    jax, fn, sh = st["jax"], st["fn"], st["sh"]

    inp_dev = jax.device_put(_prep(xyz), sh)
    prev = st["prev"]
    if prev is None:
        prev = [
            jax.device_put(np.zeros(s, d), sh)
            for s, d in zip(st["out_shape"], st["out_dtype"])
        ]
    outs = fn(inp_dev, *prev)
    res = np.asarray(outs[0])
    st["prev"] = list(outs)
    return _unpack(res)


if __name__ == "__main__":
    rng = np.random.default_rng(0)
    xyz = rng.random((8, N, 3), dtype=np.float32)
    out = kernel(xyz)
    print(out.shape, out.dtype)


# revision 16
# speedup vs baseline: 1.0001x; 1.0001x over previous
"""Ball-query kernel for Trainium2 (Bass/Tile), 8 NeuronCores.

Problem: for each batch b (8 total) and each query point m (4096), return the
first 32 source indices n (in increasing n) with ||q_m - p_n||^2 < 0.2^2,
padding unused slots with the first valid index. Queries == sources (xyz).

Sharding: data-parallel over batch, one batch per core (8 cores).

Per-core algorithm (N=4096 queries x 4096 sources):
  - PE computes dot[m, n] = q_m . p_n per 128-query block (K=3 matmul).
  - DVE scalar_tensor_tensor: s = 2*dot - plus, where plus[m,n] = sq[m]+sq[n]
    (s == -d2 with bit-exact match to the reference's rounding order).
  - DVE STT: keys = (s > -r^2) * (4096 - n)  -> valid keys descending encode
    ascending indices; invalid -> 0.
  - 4 rounds of vector.max (top-8, descending) + match_replace to extract the
    32 largest keys = first 32 valid indices, in order.
  - Pad empty slots (key 0) with the first valid key; idx = 4096 - key.

Host I/O is the bottleneck (axon-tunneled PJRT), so the host->device payload
is a single [4, 4096] f32 tensor per core (xyzT rows + sq row; sqq and the
[128, N] broadcasts are materialized on-device via rearranged/stride-0 DMA +
iota), index pairs travel back packed as 24-bit triples in a uint8 tensor
(idx_even + 4096*idx_odd, exact in f32 since it is < 2^24), and the jitted
executable + donated output buffer are cached across kernel() calls.
"""

import numpy as np

N = 4096
NS = 32
R2 = 0.04
NCORES = 8
BLK = 128
NBLK = N // BLK   # 32
CH = 2048         # dve chunk
NCH = N // CH     # 2
MM = 512          # matmul free-dim per instruction (1 bank)
NP2 = NS // 2     # 16 packed index pairs per query
NB3 = 3 * NP2     # 48 output bytes per query

_CACHE = {}


def _build_bass():
    import concourse.bass as bass
    import concourse.mybir as mybir
    from concourse import bacc, tile

    Alu = mybir.AluOpType
    f32 = mybir.dt.float32
    i32 = mybir.dt.int32
    u8 = mybir.dt.uint8

    nc = bacc.Bacc(
        "TRN2", target_bir_lowering=False, debug=False, num_devices=NCORES
    )

    # xyzT only; sq / sqq / broadcasts are derived on-device
    inp_d = nc.dram_tensor("inp", [3, N], f32, kind="ExternalInput")
    out_d = nc.dram_tensor("out", [N, NB3], u8, kind="ExternalOutput")

    with tile.TileContext(nc) as tc:
        with (
            tc.tile_pool(name="const", bufs=1) as cpool,
            tc.tile_pool(name="psum", bufs=8, space="PSUM") as ppool,
            tc.tile_pool(name="work", bufs=2) as wpool,
            tc.tile_pool(name="small", bufs=3) as spool,
        ):
            xyzT_sb = cpool.tile([3, N], f32, tag="xyzT", name="xyzT_sb")
            nc.gpsimd.dma_start(xyzT_sb[:], inp_d.ap())
            # block layout xyzB[p, 3b+c] = xyz[b*128+p, c] for per-query sq
            xyzB = cpool.tile([128, 3 * NBLK], f32, tag="xyzB", name="xyzB")
            for c in range(3):
                nc.gpsimd.dma_start(
                    xyzB[:, c::3],
                    inp_d.ap()[c : c + 1, :].rearrange("a (b p) -> p (a b)", p=128),
                )
            # sqq[p, b] = (x^2 + y^2) + z^2 of query b*128+p (IEEE order matches
            # the reference's jnp.sum on CPU)
            sqq_sb = cpool.tile([128, NBLK], f32, tag="sqq", name="sqq_sb")
            sxx = spool.tile([128, NBLK], f32, tag="sxx", name="sxx")
            syy = spool.tile([128, NBLK], f32, tag="syy", name="syy")
            szz = spool.tile([128, NBLK], f32, tag="szz", name="szz")
            nc.vector.tensor_tensor(sxx[:], xyzB[:, 0::3], xyzB[:, 0::3], Alu.mult)
            nc.vector.tensor_tensor(syy[:], xyzB[:, 1::3], xyzB[:, 1::3], Alu.mult)
            nc.vector.tensor_tensor(szz[:], xyzB[:, 2::3], xyzB[:, 2::3], Alu.mult)
            nc.vector.tensor_tensor(sxx[:], sxx[:], syy[:], Alu.add)
            nc.vector.tensor_tensor(sqq_sb[:], sxx[:], szz[:], Alu.add)
            # sqrep[p, n] = sq[n]: ones-matmul over squared coords (PE k-order
            # accumulation is bit-identical to the CPU einsum, as the dot
            # products below rely on already)
            xyzT2 = cpool.tile([3, N], f32, tag="xyzT2", name="xyzT2")
            nc.gpsimd.tensor_tensor(xyzT2[:], xyzT_sb[:], xyzT_sb[:], Alu.mult)
            ones3 = cpool.tile([3, 128], f32, tag="ones3", name="ones3")
            nc.gpsimd.memset(ones3[:], 1.0)
            sqrep = cpool.tile([128, N], f32, tag="sqrep", name="sqrep")
            for j in range(N // MM):
                ps = ppool.tile([128, MM], f32, tag="ps", name="ps")
                nc.tensor.matmul(
                    ps[:],
                    ones3[:],
                    xyzT2[:, j * MM : (j + 1) * MM],
                    start=True,
                    stop=True,
                )
                nc.vector.tensor_copy(sqrep[:, j * MM : (j + 1) * MM], ps[:])
            # ineg[p, n] = 4096 - n (exact in f32)
            ineg = cpool.tile([128, N], f32, tag="ineg", name="ineg")
            nc.gpsimd.iota(
                ineg[:],
                pattern=[[-1, N]],
                base=N,
                channel_multiplier=0,
                allow_small_or_imprecise_dtypes=True,
            )

            for b in range(NBLK):
                # plus[m, n] = sq_q[m] + sq_src[n]
                plus = wpool.tile([128, N], f32, tag="plus", name="plus")
                for c in range(NCH):
                    nc.vector.tensor_scalar(
                        plus[:, c * CH : (c + 1) * CH],
                        sqrep[:, c * CH : (c + 1) * CH],
                        sqq_sb[:, b : b + 1],
                        None,
                        Alu.add,
                    )

                keys = wpool.tile([128, N], f32, tag="keys", name="keys")
                keys2 = wpool.tile([128, N], f32, tag="keys2", name="keys2")

                for j in range(N // MM):
                    ps = ppool.tile([128, MM], f32, tag="ps", name="ps")
                    nc.tensor.matmul(
                        ps[:],
                        xyzT_sb[:, b * BLK : (b + 1) * BLK],
                        xyzT_sb[:, j * MM : (j + 1) * MM],
                        start=True,
                        stop=True,
                    )
                    # s = 2*dot - plus  (== -d2, exact)
                    nc.vector.scalar_tensor_tensor(
                        keys2[:, j * MM : (j + 1) * MM],
                        ps[:],
                        2.0,
                        plus[:, j * MM : (j + 1) * MM],
                        Alu.mult,
                        Alu.subtract,
                    )
                for c in range(NCH):
                    # keys = (s > -r2) * (4096 - n)
                    nc.vector.scalar_tensor_tensor(
                        keys[:, c * CH : (c + 1) * CH],
                        keys2[:, c * CH : (c + 1) * CH],
                        -R2,
                        ineg[:, c * CH : (c + 1) * CH],
                        Alu.is_gt,
                        Alu.mult,
                    )

                v8 = spool.tile([128, NS], f32, tag="v8", name="v8")
                nc.vector.max(v8[:, 0:8], keys[:])
                nc.vector.match_replace(keys2[:], v8[:, 0:8], keys[:], 0.0)
                nc.vector.max(v8[:, 8:16], keys2[:])
                nc.vector.match_replace(keys[:], v8[:, 8:16], keys2[:], 0.0)
                nc.vector.max(v8[:, 16:24], keys[:])
                nc.vector.match_replace(keys2[:], v8[:, 16:24], keys[:], 0.0)
                nc.vector.max(v8[:, 24:32], keys2[:])

                # pad empty slots (0) with first valid key, then pack index
                # pairs: idx_even + 4096*idx_odd = 2^24 + 4096 - k_e - 4096*k_o
                # (exact in f32: result <= 2^24 - 1)
                f8 = spool.tile([128, NS], f32, tag="f8", name="f8")
                nc.vector.tensor_scalar(f8[:], v8[:], 0.0, None, Alu.is_equal)
                t2 = spool.tile([128, NS], f32, tag="t2", name="t2")
                nc.vector.scalar_tensor_tensor(
                    t2[:], f8[:], v8[:, 0:1], v8[:], Alu.mult, Alu.add
                )
                idxf = spool.tile([128, NS], f32, tag="idxf", name="idxf")
                nc.vector.tensor_scalar(
                    idxf[:], t2[:], -1.0, float(N), Alu.mult, Alu.add
                )
                packed = spool.tile([128, NP2], i32, tag="packed", name="packed")
                nc.vector.scalar_tensor_tensor(
                    packed[:], idxf[:, NP2:NS], 4096.0, idxf[:, 0:NP2],
                    Alu.mult, Alu.add,
                )
                # byte-plane output (LE): cols 0:32 = lo16 pairs, 32:48 = hi bytes
                src_b = packed[:].bitcast(u8).rearrange("p (j k) -> p j k", k=4)
                orow = out_d.ap()[b * BLK : (b + 1) * BLK, :]
                nc.sync.dma_start(
                    orow[:, 0 : 2 * NP2].rearrange("p (j k) -> p j k", k=2),
                    src_b[:, :, 0:2],
                )
                nc.sync.dma_start(orow[:, 2 * NP2 : NB3], src_b[:, :, 2])

    nc.compile()
    return nc


def _init():
    import jax
    from jax.sharding import Mesh, PartitionSpec, NamedSharding

    try:
        from jax.experimental.shard_map import shard_map
    except ImportError:
        from jax import shard_map
    import concourse.mybir as mybir
    from concourse.bass2jax import (
        _bass_exec_p,
        install_neuronx_cc_hook,
        partition_id_tensor,
    )

    install_neuronx_cc_hook()
    nc = _build_bass()

    partition_name = (
        nc.partition_id_tensor.name if nc.partition_id_tensor else None
    )
    in_names, out_names, out_avals = [], [], []
    for alloc in nc.m.functions[0].allocations:
        if not isinstance(alloc, mybir.MemoryLocationSet):
            continue
        name = alloc.memorylocations[0].name
        if alloc.kind == "ExternalInput":
            if name != partition_name:
                in_names.append(name)
        elif alloc.kind == "ExternalOutput":
            shape = tuple(alloc.tensor_shape)
            dtype = mybir.dt.np(alloc.dtype)
            out_names.append(name)
            out_avals.append(jax.core.ShapedArray(shape, dtype))
    n_params = len(in_names)
    n_outs = len(out_avals)
    in_names_full = in_names + out_names + (
        [partition_name] if partition_name else []
    )
    donate = tuple(range(n_params, n_params + n_outs))

    def _body(*args):
        operands = list(args)
        if partition_name is not None:
            operands.append(partition_id_tensor())
        outs = _bass_exec_p.bind(
            *operands,
            out_avals=tuple(out_avals),
            in_names=tuple(in_names_full),
            out_names=tuple(out_names),
            lowering_input_output_aliases=(),
            sim_require_finite=True,
            sim_require_nnan=True,
            nc=nc,
        )
        return tuple(outs)

    devices = jax.devices()[:NCORES]
    mesh = Mesh(np.asarray(devices), ("core",))
    sh = NamedSharding(mesh, PartitionSpec("core"))
    in_specs = (PartitionSpec("core"),) * (n_params + n_outs)
    out_specs = (PartitionSpec("core"),) * n_outs
    fn = jax.jit(
        shard_map(
            _body, mesh=mesh, in_specs=in_specs, out_specs=out_specs,
            check_rep=False,
        ),
        donate_argnums=donate,
        keep_unused=True,
    )
    _CACHE.update(
        jax=jax, fn=fn, sh=sh, prev=None,
        out_shape=[(NCORES * N, NB3)], out_dtype=[np.uint8],
    )
    return _CACHE


def _prep(xyz):
    # [8, 4096, 3] -> concat of per-core xyzT [3, 4096]
    xyz = np.asarray(xyz, dtype=np.float32)
    return np.ascontiguousarray(xyz.transpose(0, 2, 1)).reshape(NCORES * 3, N)


def _unpack(res):
    # uint8 [8*4096, 48]: cols 0:32 = LE lo16 of pair j (idx_j + 4096*idx_{j+16}),
    # cols 32:48 = hi byte -> int32 [8, 4096, 32]
    lo = res[:, 0 : 2 * NP2].view(np.uint16)
    hi = res[:, 2 * NP2 : NB3]
    out = np.empty((NCORES, N, NS), np.int32)
    o = out.reshape(NCORES * N, NS)
    o[:, 0:NP2] = lo & 0xFFF
    o[:, NP2:NS] = (lo >> 12) | (hi.astype(np.uint16) << 4)
    return out


def kernel(xyz, xyz_new=None):
    st = _CACHE if _CACHE else _init()
    jax, fn, sh = st["jax"], st["fn"], st["sh"]

    inp_dev = jax.device_put(_prep(xyz), sh)
    for attempt in range(2):
        prev = st["prev"]
        if prev is None:
            # donated output-shaped operand; contents are irrelevant (the
            # kernel writes every element), fed back from the prior call
            prev = [
                jax.device_put(np.zeros(s, d), sh)
                for s, d in zip(st["out_shape"], st["out_dtype"])
            ]
        try:
            outs = fn(inp_dev, *prev)
            res = np.asarray(outs[0])
        except Exception:
            st["prev"] = None  # prev may have been consumed by donation
            if attempt:
                raise
            continue
        st["prev"] = list(outs)
        return _unpack(res)


if __name__ == "__main__":
    rng = np.random.default_rng(0)
    xyz = rng.random((8, N, 3), dtype=np.float32)
    out = kernel(xyz)
    print(out.shape, out.dtype)


# revision 19
# speedup vs baseline: 1.2856x; 1.2854x over previous
"""Ball-query kernel for Trainium2 (Bass/Tile), 8 NeuronCores.

Problem: for each batch b (8 total) and each query point m (4096), return the
first 32 source indices n (in increasing n) with ||q_m - p_n||^2 < 0.2^2,
padding unused slots with the first valid index. Queries == sources (xyz).

Sharding: data-parallel over batch, one batch per core (8 cores).

Per-core algorithm (N=4096 queries x 4096 sources):
  - PE computes dot[m, n] = q_m . p_n per 128-query block (K=3 matmul).
  - DVE scalar_tensor_tensor: s = 2*dot - plus, where plus[m,n] = sq[m]+sq[n]
    (s == -d2 with bit-exact match to the reference's rounding order).
  - DVE STT: keys = (s > -r^2) * (4096 - n)  -> valid keys descending encode
    ascending indices; invalid -> 0.
  - 4 rounds of vector.max (top-8, descending) + match_replace to extract the
    32 largest keys = first 32 valid indices, in order.
  - Pad empty slots (key 0) with the first valid key; idx = 4096 - key.

Host I/O is the bottleneck (axon-tunneled PJRT, ~73 ms per round trip plus
~20 ms/MB), so the host->device payload is just xyzT [3, 4096] f32 per core
(sq / sqq / the [128, N] broadcasts are derived on-device via strided DMA,
ones-matmul and iota), index pairs travel back packed as 24-bit values in a
byte-plane uint8 tensor (idx_j + 4096*idx_{j+16}, exact in f32 since it is
< 2^24), and the jitted executable + donated output buffer are cached across
kernel() calls so a warm call costs a single blocking round trip.
"""

import numpy as np

N = 4096
NS = 32
R2 = 0.04
NCORES = 8
BLK = 128
NBLK = N // BLK   # 32
CH = 2048         # dve chunk
NCH = N // CH     # 2
MM = 512          # matmul free-dim per instruction (1 bank)
NP2 = NS // 2     # 16 packed index pairs per query
NB3 = 3 * NP2     # 48 output bytes per query

_CACHE = {}


def _build_bass():
    import concourse.mybir as mybir
    from concourse import bacc, tile

    Alu = mybir.AluOpType
    f32 = mybir.dt.float32
    i32 = mybir.dt.int32
    u8 = mybir.dt.uint8

    nc = bacc.Bacc(
        "TRN2", target_bir_lowering=False, debug=False, num_devices=NCORES
    )

    # xyzT only; sq / sqq / broadcasts are derived on-device
    inp_d = nc.dram_tensor("inp", [3, N], f32, kind="ExternalInput")
    out_d = nc.dram_tensor("out", [N, NB3], u8, kind="ExternalOutput")

    with tile.TileContext(nc) as tc:
        with (
            tc.tile_pool(name="const", bufs=1) as cpool,
            tc.tile_pool(name="psum", bufs=8, space="PSUM") as ppool,
            tc.tile_pool(name="work", bufs=2) as wpool,
            tc.tile_pool(name="small", bufs=3) as spool,
        ):
            xyzT_sb = cpool.tile([3, N], f32, tag="xyzT", name="xyzT_sb")
            nc.gpsimd.dma_start(xyzT_sb[:], inp_d.ap())
            # block layout xyzB[p, 3b+c] = xyz[b*128+p, c] for per-query sq
            xyzB = cpool.tile([128, 3 * NBLK], f32, tag="xyzB", name="xyzB")
            for c in range(3):
                nc.gpsimd.dma_start(
                    xyzB[:, c::3],
                    inp_d.ap()[c : c + 1, :].rearrange("a (b p) -> p (a b)", p=128),
                )
            # sqq[p, b] = (x^2 + y^2) + z^2 of query b*128+p (IEEE order matches
            # the reference's jnp.sum on CPU)
            sqq_sb = cpool.tile([128, NBLK], f32, tag="sqq", name="sqq_sb")
            sxx = spool.tile([128, NBLK], f32, tag="sxx", name="sxx")
            syy = spool.tile([128, NBLK], f32, tag="syy", name="syy")
            szz = spool.tile([128, NBLK], f32, tag="szz", name="szz")
            nc.vector.tensor_tensor(sxx[:], xyzB[:, 0::3], xyzB[:, 0::3], Alu.mult)
            nc.vector.tensor_tensor(syy[:], xyzB[:, 1::3], xyzB[:, 1::3], Alu.mult)
            nc.vector.tensor_tensor(szz[:], xyzB[:, 2::3], xyzB[:, 2::3], Alu.mult)
            nc.vector.tensor_tensor(sxx[:], sxx[:], syy[:], Alu.add)
            nc.vector.tensor_tensor(sqq_sb[:], sxx[:], szz[:], Alu.add)
            # sqrep[p, n] = sq[n]: ones-matmul over squared coords (PE k-order
            # accumulation is bit-identical to the CPU einsum, as the dot
            # products below rely on already)
            xyzT2 = cpool.tile([3, N], f32, tag="xyzT2", name="xyzT2")
            nc.gpsimd.tensor_tensor(xyzT2[:], xyzT_sb[:], xyzT_sb[:], Alu.mult)
            ones3 = cpool.tile([3, 128], f32, tag="ones3", name="ones3")
            nc.gpsimd.memset(ones3[:], 1.0)
            sqrep = cpool.tile([128, N], f32, tag="sqrep", name="sqrep")
            for j in range(N // MM):
                ps = ppool.tile([128, MM], f32, tag="ps", name="ps")
                nc.tensor.matmul(
                    ps[:],
                    ones3[:],
                    xyzT2[:, j * MM : (j + 1) * MM],
                    start=True,
                    stop=True,
                )
                nc.vector.tensor_copy(sqrep[:, j * MM : (j + 1) * MM], ps[:])
            # ineg[p, n] = 4096 - n (exact in f32)
            ineg = cpool.tile([128, N], f32, tag="ineg", name="ineg")
            nc.gpsimd.iota(
                ineg[:],
                pattern=[[-1, N]],
                base=N,
                channel_multiplier=0,
                allow_small_or_imprecise_dtypes=True,
            )

            for b in range(NBLK):
                # plus[m, n] = sq_q[m] + sq_src[n]
                plus = wpool.tile([128, N], f32, tag="plus", name="plus")
                for c in range(NCH):
                    nc.vector.tensor_scalar(
                        plus[:, c * CH : (c + 1) * CH],
                        sqrep[:, c * CH : (c + 1) * CH],
                        sqq_sb[:, b : b + 1],
                        None,
                        Alu.add,
                    )

                keys = wpool.tile([128, N], f32, tag="keys", name="keys")
                keys2 = wpool.tile([128, N], f32, tag="keys2", name="keys2")

                for j in range(N // MM):
                    ps = ppool.tile([128, MM], f32, tag="ps", name="ps")
                    nc.tensor.matmul(
                        ps[:],
                        xyzT_sb[:, b * BLK : (b + 1) * BLK],
                        xyzT_sb[:, j * MM : (j + 1) * MM],
                        start=True,
                        stop=True,
                    )
                    # s = 2*dot - plus  (== -d2, exact)
                    nc.vector.scalar_tensor_tensor(
                        keys2[:, j * MM : (j + 1) * MM],
                        ps[:],
                        2.0,
                        plus[:, j * MM : (j + 1) * MM],
                        Alu.mult,
                        Alu.subtract,
                    )
                for c in range(NCH):
                    # keys = (s > -r2) * (4096 - n)
                    nc.vector.scalar_tensor_tensor(
                        keys[:, c * CH : (c + 1) * CH],
                        keys2[:, c * CH : (c + 1) * CH],
                        -R2,
                        ineg[:, c * CH : (c + 1) * CH],
                        Alu.is_gt,
                        Alu.mult,
                    )

                v8 = spool.tile([128, NS], f32, tag="v8", name="v8")
                nc.vector.max(v8[:, 0:8], keys[:])
                nc.vector.match_replace(keys2[:], v8[:, 0:8], keys[:], 0.0)
                nc.vector.max(v8[:, 8:16], keys2[:])
                nc.vector.match_replace(keys[:], v8[:, 8:16], keys2[:], 0.0)
                nc.vector.max(v8[:, 16:24], keys[:])
                nc.vector.match_replace(keys2[:], v8[:, 16:24], keys[:], 0.0)
                nc.vector.max(v8[:, 24:32], keys2[:])

                # pad empty slots (0) with first valid key, then pack index
                # pairs: idx_even + 4096*idx_odd = 2^24 + 4096 - k_e - 4096*k_o
                # (exact in f32: result <= 2^24 - 1)
                f8 = spool.tile([128, NS], f32, tag="f8", name="f8")
                nc.vector.tensor_scalar(f8[:], v8[:], 0.0, None, Alu.is_equal)
                t2 = spool.tile([128, NS], f32, tag="t2", name="t2")
                nc.vector.scalar_tensor_tensor(
                    t2[:], f8[:], v8[:, 0:1], v8[:], Alu.mult, Alu.add
                )
                idxf = spool.tile([128, NS], f32, tag="idxf", name="idxf")
                nc.vector.tensor_scalar(
                    idxf[:], t2[:], -1.0, float(N), Alu.mult, Alu.add
                )
                packed = spool.tile([128, NP2], i32, tag="packed", name="packed")
                nc.vector.scalar_tensor_tensor(
                    packed[:], idxf[:, NP2:NS], 4096.0, idxf[:, 0:NP2],
                    Alu.mult, Alu.add,
                )
                # byte-plane output (LE): cols 0:32 = lo16 pairs, 32:48 = hi bytes
                src_b = packed[:].bitcast(u8).rearrange("p (j k) -> p j k", k=4)
                orow = out_d.ap()[b * BLK : (b + 1) * BLK, :]
                nc.sync.dma_start(
                    orow[:, 0 : 2 * NP2].rearrange("p (j k) -> p j k", k=2),
                    src_b[:, :, 0:2],
                )
                nc.sync.dma_start(orow[:, 2 * NP2 : NB3], src_b[:, :, 2])

    nc.compile()
    return nc


def _init():
    import jax
    from jax.sharding import Mesh, PartitionSpec, NamedSharding

    try:
        from jax.experimental.shard_map import shard_map
    except ImportError:
        from jax import shard_map
    import concourse.mybir as mybir
    from concourse.bass2jax import (
        _bass_exec_p,
        install_neuronx_cc_hook,
        partition_id_tensor,
    )

    install_neuronx_cc_hook()
    nc = _build_bass()

    partition_name = (
        nc.partition_id_tensor.name if nc.partition_id_tensor else None
    )
    in_names, out_names, out_avals = [], [], []
    for alloc in nc.m.functions[0].allocations:
        if not isinstance(alloc, mybir.MemoryLocationSet):
            continue
        name = alloc.memorylocations[0].name
        if alloc.kind == "ExternalInput":
            if name != partition_name:
                in_names.append(name)
        elif alloc.kind == "ExternalOutput":
            shape = tuple(alloc.tensor_shape)
            dtype = mybir.dt.np(alloc.dtype)
            out_names.append(name)
            out_avals.append(jax.core.ShapedArray(shape, dtype))
    n_params = len(in_names)
    n_outs = len(out_avals)
    in_names_full = in_names + out_names + (
        [partition_name] if partition_name else []
    )
    donate = tuple(range(n_params, n_params + n_outs))

    def _body(*args):
        operands = list(args)
        if partition_name is not None:
            operands.append(partition_id_tensor())
        outs = _bass_exec_p.bind(
            *operands,
            out_avals=tuple(out_avals),
            in_names=tuple(in_names_full),
            out_names=tuple(out_names),
            lowering_input_output_aliases=(),
            sim_require_finite=True,
            sim_require_nnan=True,
            nc=nc,
        )
        return tuple(outs)

    devices = jax.devices()[:NCORES]
    mesh = Mesh(np.asarray(devices), ("core",))
    sh = NamedSharding(mesh, PartitionSpec("core"))
    in_specs = (PartitionSpec("core"),) * (n_params + n_outs)
    out_specs = (PartitionSpec("core"),) * n_outs
    fn = jax.jit(
        shard_map(
            _body, mesh=mesh, in_specs=in_specs, out_specs=out_specs,
            check_rep=False,
        ),
        donate_argnums=donate,
        keep_unused=True,
    )
    _CACHE.update(
        jax=jax, fn=fn, sh=sh, prev=None,
        out_shape=[(NCORES * N, NB3)], out_dtype=[np.uint8],
    )
    return _CACHE


def _prep(xyz):
    # [8, 4096, 3] -> concat of per-core xyzT [3, 4096]
    xyz = np.asarray(xyz, dtype=np.float32)
    return np.ascontiguousarray(xyz.transpose(0, 2, 1)).reshape(NCORES * 3, N)


def _unpack(res):
    # uint8 [8*4096, 48]: cols 0:32 = LE lo16 of pair j (idx_j + 4096*idx_{j+16}),
    # cols 32:48 = hi byte -> int32 [8, 4096, 32]
    res = np.ascontiguousarray(res)
    lo = res[:, 0 : 2 * NP2].view(np.uint16)
    hi = res[:, 2 * NP2 : NB3]
    out = np.empty((NCORES, N, NS), np.int32)
    o = out.reshape(NCORES * N, NS)
    o[:, 0:NP2] = lo & 0xFFF
    o[:, NP2:NS] = (lo >> 12) | (hi.astype(np.uint16) << 4)
    return out


def kernel(xyz, xyz_new=None):
    st = _CACHE if _CACHE else _init()
    jax, fn, sh = st["jax"], st["fn"], st["sh"]

    inp_dev = jax.device_put(_prep(xyz), sh)
    for attempt in range(2):
        prev = st["prev"]
        if prev is None:
            # donated output-shaped operand; contents are irrelevant (the
            # kernel writes every element), fed back from the prior call
            prev = [
                jax.device_put(np.zeros(s, d), sh)
                for s, d in zip(st["out_shape"], st["out_dtype"])
            ]
        try:
            outs = fn(inp_dev, *prev)
            res = np.asarray(outs[0])
        except Exception:
            st["prev"] = None  # prev may have been consumed by donation
            if attempt:
                raise
            continue
        st["prev"] = list(outs)
        return _unpack(res)


if __name__ == "__main__":
    rng = np.random.default_rng(0)
    xyz = rng.random((8, N, 3), dtype=np.float32)
    out = kernel(xyz)
    print(out.shape, out.dtype)


# revision 22
# speedup vs baseline: 1.3749x; 1.0695x over previous
"""Ball-query kernel for Trainium2 (Bass/Tile), 8 NeuronCores.

Problem: for each batch b (8 total) and each query point m (4096), return the
first 32 source indices n (in increasing n) with ||q_m - p_n||^2 < 0.2^2,
padding unused slots with the first valid index. Queries == sources (xyz).

Sharding: data-parallel over batch, one batch per core (8 cores).

Per-core algorithm (N=4096 queries x 4096 sources):
  - PE computes dot[m, n] = q_m . p_n per 128-query block (K=3 matmul).
  - DVE scalar_tensor_tensor: s = 2*dot - plus, where plus[m,n] = sq[m]+sq[n]
    (s == -d2 with bit-exact match to the reference's rounding order).
  - DVE STT: keys = (s > -r^2) * (4096 - n)  -> valid keys descending encode
    ascending indices; invalid -> 0.
  - 4 rounds of vector.max (top-8, descending) + match_replace to extract the
    32 largest keys = first 32 valid indices, in order.
  - Pad empty slots (key 0) with the first valid key; idx = 4096 - key.

Host I/O is the bottleneck (axon-tunneled PJRT, ~73 ms per round trip plus
~20 ms/MB), so the host->device payload is just xyzT [3, 4096] f32 per core
(sq / sqq / the [128, N] broadcasts are derived on-device via strided DMA,
ones-matmul and iota), index pairs travel back packed as 24-bit values in a
byte-plane uint8 tensor (idx_j + 4096*idx_{j+16}, exact in f32 since it is
< 2^24), and the jitted executable + donated output buffer are cached across
kernel() calls so a warm call costs a single blocking round trip.
"""

import numpy as np

N = 4096
NS = 32
R2 = 0.04
NCORES = 8
BLK = 128
NBLK = N // BLK   # 32
CH = 2048         # dve chunk
NCH = N // CH     # 2
MM = 512          # matmul free-dim per instruction (1 bank)
NP2 = NS // 2     # 16 packed index pairs per query
NB3 = 3 * NP2     # 48 output bytes per query

_CACHE = {}


def _build_bass():
    import concourse.mybir as mybir
    from concourse import bacc, tile

    Alu = mybir.AluOpType
    f32 = mybir.dt.float32
    i32 = mybir.dt.int32
    u8 = mybir.dt.uint8

    nc = bacc.Bacc(
        "TRN2", target_bir_lowering=False, debug=False, num_devices=NCORES
    )

    # xyzT only; sq / sqq / broadcasts are derived on-device
    inp_d = nc.dram_tensor("inp", [3, N], f32, kind="ExternalInput")
    out_d = nc.dram_tensor("out", [N, NB3], u8, kind="ExternalOutput")

    with tile.TileContext(nc) as tc:
        with (
            tc.tile_pool(name="const", bufs=1) as cpool,
            tc.tile_pool(name="psum", bufs=8, space="PSUM") as ppool,
            tc.tile_pool(name="work", bufs=2) as wpool,
            tc.tile_pool(name="small", bufs=3) as spool,
        ):
            xyzT_sb = cpool.tile([3, N], f32, tag="xyzT", name="xyzT_sb")
            nc.gpsimd.dma_start(xyzT_sb[:], inp_d.ap())
            # block layout xyzB[p, 3b+c] = xyz[b*128+p, c] for per-query sq
            xyzB = cpool.tile([128, 3 * NBLK], f32, tag="xyzB", name="xyzB")
            for c in range(3):
                nc.gpsimd.dma_start(
                    xyzB[:, c::3],
                    inp_d.ap()[c : c + 1, :].rearrange("a (b p) -> p (a b)", p=128),
                )
            # sqq[p, b] = (x^2 + y^2) + z^2 of query b*128+p (IEEE order matches
            # the reference's jnp.sum on CPU)
            sqq_sb = cpool.tile([128, NBLK], f32, tag="sqq", name="sqq_sb")
            sxx = spool.tile([128, NBLK], f32, tag="sxx", name="sxx")
            syy = spool.tile([128, NBLK], f32, tag="syy", name="syy")
            szz = spool.tile([128, NBLK], f32, tag="szz", name="szz")
            nc.vector.tensor_tensor(sxx[:], xyzB[:, 0::3], xyzB[:, 0::3], Alu.mult)
            nc.vector.tensor_tensor(syy[:], xyzB[:, 1::3], xyzB[:, 1::3], Alu.mult)
            nc.vector.tensor_tensor(szz[:], xyzB[:, 2::3], xyzB[:, 2::3], Alu.mult)
            nc.vector.tensor_tensor(sxx[:], sxx[:], syy[:], Alu.add)
            nc.vector.tensor_tensor(sqq_sb[:], sxx[:], szz[:], Alu.add)
            # sqrep[p, n] = sq[n]: ones-matmul over squared coords (PE k-order
            # accumulation is bit-identical to the CPU einsum, as the dot
            # products below rely on already)
            xyzT2 = cpool.tile([3, N], f32, tag="xyzT2", name="xyzT2")
            nc.gpsimd.tensor_tensor(xyzT2[:], xyzT_sb[:], xyzT_sb[:], Alu.mult)
            ones3 = cpool.tile([3, 128], f32, tag="ones3", name="ones3")
            nc.gpsimd.memset(ones3[:], 1.0)
            sqrep = cpool.tile([128, N], f32, tag="sqrep", name="sqrep")
            for j in range(N // MM):
                ps = ppool.tile([128, MM], f32, tag="ps", name="ps")
                nc.tensor.matmul(
                    ps[:],
                    ones3[:],
                    xyzT2[:, j * MM : (j + 1) * MM],
                    start=True,
                    stop=True,
                )
                nc.vector.tensor_copy(sqrep[:, j * MM : (j + 1) * MM], ps[:])
            # ineg[p, n] = 4096 - n (exact in f32)
            ineg = cpool.tile([128, N], f32, tag="ineg", name="ineg")
            nc.gpsimd.iota(
                ineg[:],
                pattern=[[-1, N]],
                base=N,
                channel_multiplier=0,
                allow_small_or_imprecise_dtypes=True,
            )

            for b in range(NBLK):
                # plus[m, n] = sq_q[m] + sq_src[n]
                plus = wpool.tile([128, N], f32, tag="plus", name="plus")
                for c in range(NCH):
                    nc.vector.tensor_scalar(
                        plus[:, c * CH : (c + 1) * CH],
                        sqrep[:, c * CH : (c + 1) * CH],
                        sqq_sb[:, b : b + 1],
                        None,
                        Alu.add,
                    )

                keys = wpool.tile([128, N], f32, tag="keys", name="keys")
                keys2 = wpool.tile([128, N], f32, tag="keys2", name="keys2")

                for j in range(N // MM):
                    ps = ppool.tile([128, MM], f32, tag="ps", name="ps")
                    nc.tensor.matmul(
                        ps[:],
                        xyzT_sb[:, b * BLK : (b + 1) * BLK],
                        xyzT_sb[:, j * MM : (j + 1) * MM],
                        start=True,
                        stop=True,
                    )
                    # s = 2*dot - plus  (== -d2, exact)
                    nc.vector.scalar_tensor_tensor(
                        keys2[:, j * MM : (j + 1) * MM],
                        ps[:],
                        2.0,
                        plus[:, j * MM : (j + 1) * MM],
                        Alu.mult,
                        Alu.subtract,
                    )
                for c in range(NCH):
                    # keys = (s > -r2) * (4096 - n)
                    nc.vector.scalar_tensor_tensor(
                        keys[:, c * CH : (c + 1) * CH],
                        keys2[:, c * CH : (c + 1) * CH],
                        -R2,
                        ineg[:, c * CH : (c + 1) * CH],
                        Alu.is_gt,
                        Alu.mult,
                    )

                v8 = spool.tile([128, NS], f32, tag="v8", name="v8")
                nc.vector.max(v8[:, 0:8], keys[:])
                nc.vector.match_replace(keys2[:], v8[:, 0:8], keys[:], 0.0)
                nc.vector.max(v8[:, 8:16], keys2[:])
                nc.vector.match_replace(keys[:], v8[:, 8:16], keys2[:], 0.0)
                nc.vector.max(v8[:, 16:24], keys[:])
                nc.vector.match_replace(keys2[:], v8[:, 16:24], keys[:], 0.0)
                nc.vector.max(v8[:, 24:32], keys2[:])

                # pad empty slots (0) with first valid key, then pack index
                # pairs: idx_even + 4096*idx_odd = 2^24 + 4096 - k_e - 4096*k_o
                # (exact in f32: result <= 2^24 - 1)
                f8 = spool.tile([128, NS], f32, tag="f8", name="f8")
                nc.vector.tensor_scalar(f8[:], v8[:], 0.0, None, Alu.is_equal)
                t2 = spool.tile([128, NS], f32, tag="t2", name="t2")
                nc.vector.scalar_tensor_tensor(
                    t2[:], f8[:], v8[:, 0:1], v8[:], Alu.mult, Alu.add
                )
                idxf = spool.tile([128, NS], f32, tag="idxf", name="idxf")
                nc.vector.tensor_scalar(
                    idxf[:], t2[:], -1.0, float(N), Alu.mult, Alu.add
                )
                packed = spool.tile([128, NP2], i32, tag="packed", name="packed")
                nc.vector.scalar_tensor_tensor(
                    packed[:], idxf[:, NP2:NS], 4096.0, idxf[:, 0:NP2],
                    Alu.mult, Alu.add,
                )
                # byte-plane output (LE): cols 0:32 = lo16 pairs, 32:48 = hi bytes
                src_b = packed[:].bitcast(u8).rearrange("p (j k) -> p j k", k=4)
                orow = out_d.ap()[b * BLK : (b + 1) * BLK, :]
                nc.sync.dma_start(
                    orow[:, 0 : 2 * NP2].rearrange("p (j k) -> p j k", k=2),
                    src_b[:, :, 0:2],
                )
                nc.sync.dma_start(orow[:, 2 * NP2 : NB3], src_b[:, :, 2])

    nc.compile()
    return nc


def _init():
    import jax
    from jax.sharding import Mesh, PartitionSpec, NamedSharding

    try:
        from jax.experimental.shard_map import shard_map
    except ImportError:
        from jax import shard_map
    import concourse.mybir as mybir
    from concourse.bass2jax import (
        _bass_exec_p,
        install_neuronx_cc_hook,
        partition_id_tensor,
    )

    install_neuronx_cc_hook()
    nc = _build_bass()

    partition_name = (
        nc.partition_id_tensor.name if nc.partition_id_tensor else None
    )
    in_names, out_names, out_avals = [], [], []
    for alloc in nc.m.functions[0].allocations:
        if not isinstance(alloc, mybir.MemoryLocationSet):
            continue
        name = alloc.memorylocations[0].name
        if alloc.kind == "ExternalInput":
            if name != partition_name:
                in_names.append(name)
        elif alloc.kind == "ExternalOutput":
            shape = tuple(alloc.tensor_shape)
            dtype = mybir.dt.np(alloc.dtype)
            out_names.append(name)
            out_avals.append(jax.core.ShapedArray(shape, dtype))
    n_params = len(in_names)
    n_outs = len(out_avals)
    in_names_full = in_names + out_names + (
        [partition_name] if partition_name else []
    )
    donate = tuple(range(n_params, n_params + n_outs))

    def _body(*args):
        operands = list(args)
        if partition_name is not None:
            operands.append(partition_id_tensor())
        outs = _bass_exec_p.bind(
            *operands,
            out_avals=tuple(out_avals),
            in_names=tuple(in_names_full),
            out_names=tuple(out_names),
            lowering_input_output_aliases=(),
            sim_require_finite=True,
            sim_require_nnan=True,
            nc=nc,
        )
        return tuple(outs)

    devices = jax.devices()[:NCORES]
    mesh = Mesh(np.asarray(devices), ("core",))
    sh = NamedSharding(mesh, PartitionSpec("core"))
    in_specs = (PartitionSpec("core"),) * (n_params + n_outs)
    out_specs = (PartitionSpec("core"),) * n_outs
    fn = jax.jit(
        shard_map(
            _body, mesh=mesh, in_specs=in_specs, out_specs=out_specs,
            check_rep=False,
        ),
        donate_argnums=donate,
        keep_unused=True,
    )
    from concurrent.futures import ThreadPoolExecutor

    _CACHE.update(
        jax=jax, fn=fn, sh=sh, prev=None,
        out_shape=[(NCORES * N, NB3)], out_dtype=[np.uint8],
        pool=ThreadPoolExecutor(NCORES),
    )
    return _CACHE


def _prep(xyz):
    # [8, 4096, 3] -> concat of per-core xyzT [3, 4096]
    xyz = np.asarray(xyz, dtype=np.float32)
    return np.ascontiguousarray(xyz.transpose(0, 2, 1)).reshape(NCORES * 3, N)


def _unpack_into(res, o):
    # uint8 [4096, 48]: cols 0:32 = LE lo16 of pair j (idx_j + 4096*idx_{j+16}),
    # cols 32:48 = hi byte -> int32 [4096, 32] written into o
    res = np.ascontiguousarray(res)
    lo = res[:, 0 : 2 * NP2].view(np.uint16)
    hi = res[:, 2 * NP2 : NB3]
    o[:, 0:NP2] = lo & 0xFFF
    o[:, NP2:NS] = (lo >> 12) | (hi.astype(np.uint16) << 4)


def _fetch(outs, pool):
    # fetch the 8 per-core shards concurrently, unpacking each as it lands
    # (overlaps host decode with the remaining transfers)
    out = np.empty((NCORES, N, NS), np.int32)

    def one(shard):
        core = shard.index[0].start // N if shard.index[0].start else 0
        _unpack_into(np.asarray(shard.data), out[core])

    list(pool.map(one, outs[0].addressable_shards))
    return out


def kernel(xyz, xyz_new=None):
    st = _CACHE if _CACHE else _init()
    jax, fn, sh = st["jax"], st["fn"], st["sh"]

    inp_dev = jax.device_put(_prep(xyz), sh)
    for attempt in range(2):
        prev = st["prev"]
        if prev is None:
            # donated output-shaped operand; contents are irrelevant (the
            # kernel writes every element), fed back from the prior call
            prev = [
                jax.device_put(np.zeros(s, d), sh)
                for s, d in zip(st["out_shape"], st["out_dtype"])
            ]
        try:
            outs = fn(inp_dev, *prev)
            res = _fetch(outs, st["pool"])
        except Exception:
            st["prev"] = None  # prev may have been consumed by donation
            if attempt:
                raise
            continue
        st["prev"] = list(outs)
        return res


if __name__ == "__main__":
    rng = np.random.default_rng(0)
    xyz = rng.random((8, N, 3), dtype=np.float32)
    out = kernel(xyz)
    print(out.shape, out.dtype)
